# revision 8
# baseline (speedup 1.0000x reference)
"""ChannelAttention kernel for Trainium2, 8-core SPMD.

Reference computation (per batch b):
    G   = X @ X.T                 # [C, C], X = x[b] reshaped [C, H*W]
    att = softmax(G, axis=0)      # softmax over first (query-channel) axis
    xw  = att.T? -- precisely: xw[c, n] = sum_d att[c, d] * X[d, n]
    out = beta * xw + x

Sharding: 8 cores = (batch b in 0..3) x (half h in 0..1 of the N = H*W axis).
Each core streams its [C=128, N/2=32768] fp32 slice in once, builds the
partial Gram matrix G_h = X_h @ X_h.T on the PE (bf16 operands, fp32
accumulate), pairs (2b, 2b+1) AllReduce the 64 KiB partial G, softmax runs
along the free axis (G is symmetric, so the [d, c]-layout softmax gives
att.T directly, which is exactly the stationary operand the second GEMM
needs), and the core computes out = x + beta * (att.T.T @ X) for its own
N-half and streams it out.  Total HBM traffic per core: 16 MiB in + 16 MiB
out + 128 KiB collective bounce.
"""

import numpy as np

import concourse.bacc as bacc
import concourse.mybir as mybir
import concourse.tile as tile
from concourse.bass_utils import run_bass_kernel_spmd
from concourse.masks import make_identity

# Problem geometry (hardcoded per contract).
B, C, H, W = 4, 128, 256, 256
N_FULL = H * W              # 65536
N_CORES = 8
HALVES = 2
N = N_FULL // HALVES        # 32768 per core
P = 128                     # partitions == C
CHUNK = 2048                # fp32 DMA chunk (1 MiB per transfer)
NCHUNK = N // CHUNK         # 16
TT = 128                    # transpose tile width
MM_N = 512                  # second-GEMM moving free size (one PSUM bank fp32)

F32 = mybir.dt.float32
BF16 = mybir.dt.bfloat16


def build_nc(n_iters: int = 1, mock_cc: bool = False):
    """Build the 8-core SPMD Bass program. n_iters>1 wraps the whole body in
    a hardware loop; mock_cc replaces the AllReduce with a DRAM copy (both
    for benchmarking/modeling only)."""
    nc = bacc.Bacc("TRN2", target_bir_lowering=False, debug=False,
                   num_devices=N_CORES)

    x_d = nc.dram_tensor("x", [P, N], F32, kind="ExternalInput")
    beta_d = nc.dram_tensor("betab", [P, 1], F32, kind="ExternalInput")
    out_d = nc.dram_tensor("out", [P, N], F32, kind="ExternalOutput")

    groups = [[2 * b, 2 * b + 1] for b in range(B)]

    with tile.TileContext(nc) as tc:
        with (
            tc.tile_pool(name="const", bufs=1) as constp,
            tc.tile_pool(name="xbuf", bufs=1) as xp,
            tc.tile_pool(name="x16buf", bufs=1) as x16p,
            tc.tile_pool(name="xt", bufs=4) as xtp,
            tc.tile_pool(name="small", bufs=1) as smallp,
            tc.tile_pool(name="pt", bufs=3, space="PSUM") as ptp,
            tc.tile_pool(name="pg", bufs=1, space="PSUM") as pgp,
            tc.tile_pool(name="pw", bufs=3, space="PSUM") as pwp,
            tc.tile_pool(name="dram", bufs=1, space="DRAM") as dramp,
        ):
            ident = constp.tile([P, P], BF16, tag="ident")
            make_identity(nc, ident[:])
            beta_s = constp.tile([P, 1], F32, tag="beta")
            nc.sync.dma_start(beta_s[:], beta_d[:, :])

            def body(_i=None):
                xch = []
                x16ch = []
                gidx = 0
                n_gram = N // TT
                QUAD = 4 * TT  # four 128-wide transposes share one PSUM bank
                g_ps = pgp.tile([P, P], F32, tag="G")
                for ch in range(NCHUNK):
                    xt_ = xp.tile([P, CHUNK], F32, tag=f"x{ch}")
                    nc.sync.dma_start(xt_[:], x_d[:, ch * CHUNK:(ch + 1) * CHUNK])
                    x16 = x16p.tile([P, CHUNK], BF16, tag=f"h{ch}")
                    nc.scalar.copy(x16[:], xt_[:])
                    xch.append(xt_)
                    x16ch.append(x16)
                    for q in range(CHUNK // QUAD):
                        pt = ptp.tile([P, QUAD], BF16, tag="pt")
                        for t in range(4):
                            nc.tensor.transpose(
                                pt[:, t * TT:(t + 1) * TT],
                                x16[:, q * QUAD + t * TT:q * QUAD + (t + 1) * TT],
                                ident[:])
                        xtile = xtp.tile([P, QUAD], BF16, tag="xt")
                        nc.vector.tensor_copy(xtile[:], pt[:])
                        for t in range(4):
                            sl = slice(t * TT, (t + 1) * TT)
                            nc.tensor.matmul(g_ps[:], lhsT=xtile[:, sl],
                                             rhs=xtile[:, sl],
                                             start=(gidx == 0),
                                             stop=(gidx == n_gram - 1))
                            gidx += 1

                # Pair AllReduce of the partial Gram matrix through DRAM.
                g_sb = smallp.tile([P, P], F32, tag="gsb")
                nc.vector.tensor_copy(g_sb[:], g_ps[:])
                cc_in = dramp.tile([P, P], F32, tag="ccin")
                cc_out = dramp.tile([P, P], F32, tag="ccout")
                nc.sync.dma_start(cc_in[:], g_sb[:])
                if mock_cc:
                    nc.sync.dma_start(cc_out[:], cc_in[:])
                else:
                    nc.gpsimd.collective_compute(
                        "AllReduce", mybir.AluOpType.add, replica_groups=groups,
                        ins=[cc_in.opt()], outs=[cc_out.opt()],
                    )
                g_full = smallp.tile([P, P], F32, tag="gfull")
                nc.sync.dma_start(g_full[:], cc_out[:])

                # Softmax along the free axis: tile viewed as [d, c] (G is
                # symmetric) -> result is att.T in [d, c] layout.
                mx = smallp.tile([P, 1], F32, tag="mx")
                nc.vector.reduce_max(mx[:], g_full[:], axis=mybir.AxisListType.X)
                nmx = smallp.tile([P, 1], F32, tag="nmx")
                nc.vector.tensor_scalar_mul(nmx[:], mx[:], -1.0)
                esum = smallp.tile([P, 1], F32, tag="esum")
                eexp = smallp.tile([P, P], F32, tag="eexp")
                nc.scalar.activation(eexp[:], g_full[:],
                                     mybir.ActivationFunctionType.Exp,
                                     bias=nmx[:], scale=1.0, accum_out=esum[:])
                rsum = smallp.tile([P, 1], F32, tag="rsum")
                nc.vector.reciprocal(rsum[:], esum[:])
                # attT scaled by beta/rowsum in one pass: (beta*att).T in bf16.
                # Exact when beta == 0 (everything downstream is exactly 0).
                attT = smallp.tile([P, P], BF16, tag="attT")
                nc.vector.tensor_scalar(out=attT[:], in0=eexp[:],
                                        scalar1=rsum[:], scalar2=beta_s[:],
                                        op0=mybir.AluOpType.mult,
                                        op1=mybir.AluOpType.mult)

                # out = x + (beta*att).T.T @ X16  streamed back out per chunk.
                for ch in range(NCHUNK):
                    xt_ = xch[ch]
                    x16 = x16ch[ch]
                    for s in range(CHUNK // MM_N):
                        pw = pwp.tile([P, MM_N], F32, tag="pw")
                        nc.tensor.matmul(pw[:], lhsT=attT[:],
                                         rhs=x16[:, s * MM_N:(s + 1) * MM_N],
                                         start=True, stop=True)
                        sl = slice(s * MM_N, (s + 1) * MM_N)
                        nc.vector.tensor_tensor(out=xt_[:, sl], in0=xt_[:, sl],
                                                in1=pw[:],
                                                op=mybir.AluOpType.add)
                    nc.sync.dma_start(out_d[:, ch * CHUNK:(ch + 1) * CHUNK],
                                      xt_[:])

            if n_iters == 1:
                body()
            else:
                with tc.For_i(0, n_iters, 1) as i:
                    body(i)

    nc.compile()
    return nc


_CACHE = {}


def _get_nc(n_iters: int = 1):
    if n_iters not in _CACHE:
        _CACHE[n_iters] = build_nc(n_iters)
    return _CACHE[n_iters]


def make_in_maps(x: np.ndarray, beta: np.ndarray):
    xr = x.reshape(B, C, N_FULL)
    betab = np.ascontiguousarray(
        np.broadcast_to(beta.reshape(1, 1), (P, 1)).astype(np.float32))
    in_maps = []
    for core in range(N_CORES):
        b, h = divmod(core, HALVES)
        # views are fine: run_bass_via_pjrt concatenates (and thus copies)
        in_maps.append({"x": xr[b, :, h * N:(h + 1) * N], "betab": betab})
    return in_maps


def assemble_out(results):
    out = np.empty((B, C, N_FULL), dtype=np.float32)
    for core in range(N_CORES):
        b, h = divmod(core, HALVES)
        out[b, :, h * N:(h + 1) * N] = results[core]["out"]
    return out.reshape(B, C, H, W)


def kernel(x: np.ndarray, beta: np.ndarray) -> np.ndarray:
    nc = _get_nc(1)
    in_maps = make_in_maps(np.asarray(x, dtype=np.float32),
                           np.asarray(beta, dtype=np.float32))
    last_err = None
    for attempt in range(3):
        try:
            res = run_bass_kernel_spmd(nc, in_maps,
                                       core_ids=list(range(N_CORES)))
            return assemble_out(res.results)
        except Exception as e:  # transient device/tunnel failures observed
            last_err = e
            import time as _time
            try:
                import jax as _jax
                _jax.clear_caches()
            except Exception:
                pass
            _time.sleep(10.0 * (attempt + 1))
    raise last_err
